# revision 6
# baseline (speedup 1.0000x reference)
"""Trainium2 Bass kernel for nn_BilateralFilter (exact Gaussian bilateral filter).

Math (per reference):
  feats f_i in R^6 (scaled spatial zyx + scaled rgb), N = 12*24*24 = 6912
  sq[i,j] = |f_i - f_j|^2 ;  K = exp(-0.5*sq)
  out[c,j] = (sum_i q[c,i] K[i,j]) / (sum_i K[i,j] + eps)

Device strategy (8 cores, row-sharded over the N x N kernel): each core owns
J = 864 output columns.  Per 128-i tile: matmul1 computes
arg = f_i.f_j - 0.5|f_i|^2 - 0.5|f_j|^2 = -0.5*sq in PSUM via an augmented
bf16 hi/mid/lo-split contraction (K=48 real rows in a 64-row strip), the exp
runs on ScalarE (exact LUT, even tiles) or VectorE (Schraudolph bf16
bit-trick: bf16(exp(x)) ~ bitcast_int16(round(x*128/ln2 + 127*128 - c)),
odd tiles), and matmul2 contracts K against [q_hi(2), q_lo(2), ones] bf16
columns into a PSUM accumulator.

PE efficiency tricks (all HW-verified):
  - matmul1 row-pairing: even tiles occupy PE rows 0-63, odd tiles rows
    64-127 (tile_position row groups), so consecutive tiles' matmuls run
    concurrently in the array.
  - matmul2 col-grouping: M=5 outputs only; tile t targets col strip
    32*(t%4), so 4 consecutive tiles' matmul2s run concurrently.  All four
    groups share one PSUM accumulator region (has_written clear is
    per-element), host sums the 4 partials.
  - matmul2 batches are emitted 4+ tiles behind the producing exp so the PE
    never stalls the exp engines.

Host does only O(N) layout/prep and the final group-fold + normalize.
"""

import numpy as np

try:
    import concourse.bass as bass
except ImportError:  # fresh grading dir: repo not on sys.path
    import sys

    sys.path.insert(0, "/opt/trn_rl_repo")
    import concourse.bass as bass

import concourse.mybir as mybir
import concourse.tile as tile
from concourse import bacc
from concourse.bass_utils import run_bass_kernel_spmd

SIGMA_ALPHA = (5.0, 5.0, 5.0)
SIGMA_BETA = 0.3
EPS = float(np.finfo("float").eps)

D, H, W = 12, 24, 24
N = D * H * W  # 6912
M_CORES = 8
J = N // M_CORES  # 864 output columns per core
NT = N // 128  # 54 i-tiles
F = 8  # augmented feature dim
J_CHUNKS = [(0, 512), (512, 864)]  # matmul free-dim chunks, PSUM-bank aligned
NGRP = 4  # matmul2 col groups

# Schraudolph constants for bf16-domain exp via int16 round-to-nearest:
#   bits = rint(x * 2^7/ln2 + (127*2^7 - C));  bf16 <- int16 bits
# C tuned numerically end-to-end (error is a sawtooth in frac(x*log2e) that
# largely cancels between the filtered numerator and the norm denominator).
SCHRAUD_SCALE = 184.6650390625
SCHRAUD_BIAS = 16256.0 - 8.0

_BUILD_CACHE: dict[str, object] = {}


def _grp_tiles(g):
    return list(range(g, NT, NGRP))


def _build_nc():
    bf16 = mybir.dt.bfloat16
    f32 = mybir.dt.float32
    nc = bacc.Bacc(None, target_bir_lowering=False)

    a_dram = nc.dram_tensor("a48", [128, N], bf16, kind="ExternalInput")
    b_dram = nc.dram_tensor("b48", [128, J], bf16, kind="ExternalInput")
    qa_dram = nc.dram_tensor("qa5", [128, NT * 5], bf16, kind="ExternalInput")
    out_dram = nc.dram_tensor("acc_out", [101, J], f32, kind="ExternalOutput")

    E = mybir.ActivationFunctionType.Exp
    MU, AD = mybir.AluOpType.mult, mybir.AluOpType.add

    with tile.TileContext(nc) as tc:
        with (
            tc.tile_pool(name="const", bufs=1) as const_pool,
            tc.tile_pool(name="kpool", bufs=12) as kpool,
            tc.tile_pool(name="gring", bufs=1, space="PSUM") as gring_pool,
            tc.tile_pool(name="apsum", bufs=1, space="PSUM") as apool,
            tc.tile_pool(name="opool", bufs=1) as opool,
        ):
            A48 = const_pool.tile([128, N], bf16)
            B48 = const_pool.tile([128, J], bf16)
            QA5 = const_pool.tile([128, NT * 5], bf16)

            # Prewarm the ACT exp table set (~2.7us) during the DMA window.
            warm = const_pool.tile([1, 8], f32)
            kw = const_pool.tile([1, 8], bf16)
            nc.vector.memset(warm[:], -1.0)
            nc.scalar.activation(kw[:], warm[:], E)

            # Input DMAs, ordered so the first pair's dependencies land first.
            # All transfers are per-partition contiguous (no tiny-packet APs).
            nc.sync.dma_start(A48[:, 0:256], a_dram[:, 0:256])
            nc.sync.dma_start(B48[:], b_dram[:])
            nc.sync.dma_start(QA5[:], qa_dram[:])
            nc.sync.dma_start(A48[:, 256:2048], a_dram[:, 256:2048])
            nc.sync.dma_start(A48[:, 2048:N], a_dram[:, 2048:N])

            # col group g accumulates at partitions 32g..32g+4.  Nothing else
            # may write these banks: a start=True matmul clears has_written
            # for its partition rows across the whole bank (dropping real
            # partials), and a start=False one costs PSUM read-modify-write
            # activity that trips the PE power throttle.
            acc = apool.tile([101, J], f32)

            # arg tiles: a manually-managed ring of 3 persistent slots (2 PSUM
            # banks each).  Persistent tiles (not a cycling pool) make Tile
            # track dependencies at byte-range granularity: the spare cols
            # 864:1024 of each slot form a dependency-free scratch target for
            # HAM filler matmuls.  start=True is harmless here -- g is only
            # ever written with full-overwrite semantics.
            ring = [
                gring_pool.tile([128, 1024], f32, name=f"gslot{i}") for i in range(3)
            ]

            # HAM warmup: ~4us of back-to-back dummy matmuls during the DMA
            # window releases the PE clock throttle (1.2 -> 2.4 GHz) before
            # real work starts; the per-pair fillers below then keep it warm.
            dum = const_pool.tile([128, 256], bf16)
            nc.gpsimd.memset(dum[:], 0.5)
            for _ in range(12):
                nc.tensor.matmul(
                    ring[0][:, 864:1024], dum[:, 0:128], dum[:, 0:160],
                    start=True, stop=True, skip_group_check=True,
                )

            grp_last = {g: _grp_tiles(g)[-1] for g in range(NGRP)}
            kts: list = [None] * NT
            pending: list[int] = []  # tiles whose matmul2 is not yet emitted

            def emit_mm2(tiles):
                for j0, j1 in J_CHUNKS:
                    for t in tiles:
                        g = t % NGRP
                        k = kts[t]
                        rhs = k[:, j0:j1]
                        if k.tensor.dtype == mybir.dt.int16:
                            rhs = rhs.bitcast(bf16)
                        nc.tensor.matmul(
                            acc[32 * g : 32 * g + 5, j0:j1],
                            QA5[:, 5 * t : 5 * t + 5],
                            rhs,
                            start=(t == g),
                            stop=(t == grp_last[g]),
                            skip_group_check=True,
                            tile_position=(0, 32 * g),
                        )
                for t in tiles:
                    kts[t] = None

            for p in range(NT // 2):  # pair (2p, 2p+1): rows 0-63 / 64-127
                t0, t1 = 2 * p, 2 * p + 1
                g0 = ring[(2 * p) % 3]
                g1 = ring[(2 * p + 1) % 3]
                for j0, j1 in J_CHUNKS:
                    nc.tensor.matmul(
                        g0[:, j0:j1],
                        A48[0:64, t0 * 128 : (t0 + 1) * 128],
                        B48[0:64, j0:j1],
                        start=True,
                        stop=True,
                    )
                    nc.tensor.matmul(
                        g1[:, j0:j1],
                        A48[64:128, t1 * 128 : (t1 + 1) * 128],
                        B48[64:128, j0:j1],
                        start=True,
                        stop=True,
                    )
                # exp: ScalarE takes the even tile, VectorE the odd tile.
                # ScalarE is slightly faster per tile, so one pair mid-stream
                # goes all-ScalarE to balance total busy time (28/26 split).
                k0 = kpool.tile([128, J], bf16)
                nc.scalar.activation(k0[:], g0[:, 0:J], E)
                kts[t0] = k0
                if p == 13:
                    k1 = kpool.tile([128, J], bf16, name="k1a", tag="k1a", bufs=2)
                    nc.scalar.activation(k1[:], g1[:, 0:J], E)
                else:
                    k1 = kpool.tile([128, J], mybir.dt.int16, name="k1", tag="k1")
                    nc.vector.tensor_scalar(
                        k1[:], g1[:, 0:J], SCHRAUD_SCALE, SCHRAUD_BIAS, MU, AD
                    )
                kts[t1] = k1
                # HAM filler: the PE is only ~60% busy at the exp-paced
                # cadence, and the clock throttle re-engages unless the PE
                # stream is near gap-free.  Two throwaway matmuls per pair pad
                # the idle slivers; the ring-slot spare cols have no readers,
                # so these run whenever the PE would otherwise stall.
                for _ in range(2):
                    nc.tensor.matmul(
                        ring[(2 * p) % 3][:, 864:1024], dum[:, 0:128], dum[:, 0:160],
                        start=True, stop=True, skip_group_check=True,
                    )
                pending += [t0, t1]
                # Emit matmul2 for tiles at least 4 behind: their k tiles are
                # long since ready, so these never stall the PE queue (a
                # stalled matmul2 blocks the whole PE FIFO behind it, the PE
                # goes idle, and the clock throttle re-engages).
                if len(pending) >= 8:
                    emit_mm2(pending[:4])
                    pending = pending[4:]

            while pending:
                emit_mm2(pending[:4])
                pending = pending[4:]

            out_sb = opool.tile([101, J], f32)
            nc.scalar.copy(out_sb[:], acc[0:101, 0:J])
            nc.sync.dma_start(out_dram[:], out_sb[:])

    nc.compile()
    return nc


def _get_nc():
    nc = _BUILD_CACHE.get("v2")
    if nc is None:
        nc = _build_nc()
        _BUILD_CACHE["v2"] = nc
    return nc


def _split_bf16_3(a):
    import ml_dtypes

    bf = ml_dtypes.bfloat16
    a = np.asarray(a, dtype=np.float32)
    h = a.astype(bf)
    m = (a - h.astype(np.float32)).astype(bf)
    l = (a - h.astype(np.float32) - m.astype(np.float32)).astype(bf)
    return h, m, l


def _host_prep(q_in, image, v_alpha, v_beta):
    """Augmented feature matrices (fp32, O(N) work only)."""
    q_in = np.asarray(q_in, dtype=np.float32)
    image = np.asarray(image, dtype=np.float32)
    v_alpha = np.asarray(v_alpha, dtype=np.float32)
    v_beta = np.asarray(v_beta, dtype=np.float32)

    z = np.arange(D, dtype=np.float32)[:, None, None]
    y = np.arange(H, dtype=np.float32)[None, :, None]
    x = np.arange(W, dtype=np.float32)[None, None, :]
    shp = (D, H, W)
    zz = np.broadcast_to(v_alpha[0] * z / np.float32(SIGMA_ALPHA[0]), shp)
    xx = np.broadcast_to(v_alpha[1] * x / np.float32(SIGMA_ALPHA[1]), shp)
    yy = np.broadcast_to(v_alpha[2] * y / np.float32(SIGMA_ALPHA[2]), shp)
    xyz = np.stack([zz, yy, xx], axis=3)
    rgb = v_beta * np.transpose(image, (1, 2, 3, 0)) / np.float32(SIGMA_BETA)
    feats = np.concatenate([xyz, rgb], axis=3).reshape(-1, 6).astype(np.float32)

    # Center each feature dim: |f_i - f_j| is translation invariant, smaller
    # magnitudes mean less cancellation in the PE accumulation.
    feats = feats - (feats.min(axis=0) + feats.max(axis=0)) * np.float32(0.5)

    s = np.einsum("nf,nf->n", feats, feats).astype(np.float32)

    a_all = np.empty((F, N), dtype=np.float32)
    a_all[0:6] = feats.T
    a_all[6] = -0.5 * s
    a_all[7] = 1.0

    b_full = np.empty((F, N), dtype=np.float32)
    b_full[0:6] = feats.T
    b_full[6] = 1.0
    b_full[7] = -0.5 * s

    qa = np.empty((N, 3), dtype=np.float32)
    qa[:, 0] = q_in[0].reshape(-1)
    qa[:, 1] = q_in[1].reshape(-1)
    qa[:, 2] = 1.0
    return a_all, b_full, qa


def _in_maps(a_all, b_full, qa):
    """Per-core input dict: split/stacked bf16 layouts.

    a48: K=48 stacked splits [Ah;Ah;Ah;Am;Am;Al] in a 64-row strip; even
    tiles at partitions 0-63, odd tiles at 64-127 (row-paired matmul1).
    b48: [Bh;Bm;Bl;Bh;Bm;Bh] splits duplicated into both 64-row strips.
    qa5: [qh0,qh1,qm0,qm1,ones] per tile, on-chip layout [128, NT*5].
    """
    import ml_dtypes

    bf = ml_dtypes.bfloat16
    ah, am, al = _split_bf16_3(a_all)
    bh, bm, bl = _split_bf16_3(b_full)
    a48 = np.concatenate([ah, ah, ah, am, am, al], axis=0)  # [48, N]
    b48s = np.concatenate([bh, bm, bl, bh, bm, bh], axis=0)  # [48, N]

    a128 = np.zeros((128, N), dtype=bf)
    for t in range(NT):
        cs = slice(t * 128, (t + 1) * 128)
        a128[(t % 2) * 64 : (t % 2) * 64 + 48, cs] = a48[:, cs]

    qh = qa[:, 0:2].astype(bf)
    qm = (qa[:, 0:2] - qh.astype(np.float32)).astype(bf)
    qa5 = np.empty((N, 5), dtype=bf)
    qa5[:, 0:2] = qh
    qa5[:, 2:4] = qm
    qa5[:, 4] = np.float32(1.0)
    # on-chip layout: partition p, column 5t+c  <-  row t*128+p, col c
    qa5_chip = np.ascontiguousarray(
        qa5.reshape(NT, 128, 5).transpose(1, 0, 2).reshape(128, NT * 5)
    )

    maps = []
    for c in range(M_CORES):
        bslab = b48s[:, c * J : (c + 1) * J]
        b128 = np.zeros((128, J), dtype=bf)
        b128[0:48] = bslab
        b128[64:112] = bslab
        maps.append({"a48": a128, "b48": b128, "qa5": qa5_chip})
    return maps


def _fold_output(res):
    accs = []
    for c in range(M_CORES):
        o = res.results[c]["acc_out"]  # [101, J]; group g at rows 32g..32g+4
        accs.append(sum(o[32 * g : 32 * g + 5] for g in range(NGRP)))
    acc = np.concatenate(accs, axis=1)  # [5, N]
    filtered = acc[0:2] + acc[2:4]
    norm = acc[4]
    out = filtered / (norm[None, :] + EPS)
    return out.reshape(2, D, H, W).astype(np.float32)


def kernel(q_in, image, v_alpha, v_beta):
    a_all, b_full, qa = _host_prep(q_in, image, v_alpha, v_beta)
    nc = _get_nc()
    in_maps = _in_maps(a_all, b_full, qa)
    res = run_bass_kernel_spmd(nc, in_maps, core_ids=list(range(M_CORES)))
    return _fold_output(res)


# revision 7
# speedup vs baseline: 1.5150x; 1.5150x over previous
"""Trainium2 Bass kernel for nn_BilateralFilter (exact Gaussian bilateral filter).

Math (per reference):
  feats f_i in R^6 (scaled spatial zyx + scaled rgb), N = 12*24*24 = 6912
  sq[i,j] = |f_i - f_j|^2 ;  K = exp(-0.5*sq)
  out[c,j] = (sum_i q[c,i] K[i,j]) / (sum_i K[i,j] + eps)

Device strategy (8 cores, row-sharded over the N x N kernel): each core owns
J = 864 output columns.  Per 128-i tile: matmul1 computes
arg = f_i.f_j - 0.5|f_i|^2 - 0.5|f_j|^2 = -0.5*sq in PSUM via an augmented
bf16 hi/mid/lo-split contraction (K=48 real rows in a 64-row strip), the exp
runs on ScalarE (exact LUT, even tiles) or VectorE (Schraudolph bf16
bit-trick: bf16(exp(x)) ~ bitcast_int16(round(x*128/ln2 + 127*128 - c)),
odd tiles), and matmul2 contracts K against [q_hi(2), q_lo(2), ones] bf16
columns into a PSUM accumulator.

PE efficiency tricks (all HW-verified):
  - matmul1 row-pairing: even tiles occupy PE rows 0-63, odd tiles rows
    64-127 (tile_position row groups), so consecutive tiles' matmuls run
    concurrently in the array.
  - matmul2 col-grouping: M=5 outputs only; tile t targets col strip
    32*(t%4), so 4 consecutive tiles' matmul2s run concurrently.  All four
    groups share one PSUM accumulator region (has_written clear is
    per-element), host sums the 4 partials.
  - matmul2 batches are emitted 4+ tiles behind the producing exp so the PE
    never stalls the exp engines.

Host does only O(N) layout/prep and the final group-fold + normalize.
"""

import numpy as np

try:
    import concourse.bass as bass
except ImportError:  # fresh grading dir: repo not on sys.path
    import sys

    sys.path.insert(0, "/opt/trn_rl_repo")
    import concourse.bass as bass

import concourse.mybir as mybir
import concourse.tile as tile
from concourse import bacc
from concourse.bass_utils import run_bass_kernel_spmd

SIGMA_ALPHA = (5.0, 5.0, 5.0)
SIGMA_BETA = 0.3
EPS = float(np.finfo("float").eps)

D, H, W = 12, 24, 24
N = D * H * W  # 6912
M_CORES = 8
J = N // M_CORES  # 864 output columns per core
NT = N // 128  # 54 i-tiles
F = 8  # augmented feature dim
J_CHUNKS = [(0, 512), (512, 864)]  # matmul free-dim chunks, PSUM-bank aligned
NGRP = 4  # matmul2 col groups

# Schraudolph constants for bf16-domain exp via int16 round-to-nearest:
#   bits = rint(x * 2^7/ln2 + (127*2^7 - C));  bf16 <- int16 bits
# C tuned numerically end-to-end (error is a sawtooth in frac(x*log2e) that
# largely cancels between the filtered numerator and the norm denominator).
SCHRAUD_SCALE = 184.6650390625
SCHRAUD_BIAS = 16256.0 - 8.0

_BUILD_CACHE: dict[str, object] = {}


def _grp_tiles(g):
    return list(range(g, NT, NGRP))


def _build_nc():
    bf16 = mybir.dt.bfloat16
    f32 = mybir.dt.float32
    nc = bacc.Bacc(None, target_bir_lowering=False)

    a_dram = nc.dram_tensor("a48", [128, N], bf16, kind="ExternalInput")
    b_dram = nc.dram_tensor("b48", [128, J], bf16, kind="ExternalInput")
    qa_dram = nc.dram_tensor("qa5", [128, NT * 5], bf16, kind="ExternalInput")
    out_dram = nc.dram_tensor("acc_out", [101, J], f32, kind="ExternalOutput")

    E = mybir.ActivationFunctionType.Exp
    MU, AD = mybir.AluOpType.mult, mybir.AluOpType.add

    with tile.TileContext(nc) as tc:
        with (
            tc.tile_pool(name="const", bufs=1) as const_pool,
            tc.tile_pool(name="kpool", bufs=12) as kpool,
            tc.tile_pool(name="gring", bufs=1, space="PSUM") as gring_pool,
            tc.tile_pool(name="apsum", bufs=1, space="PSUM") as apool,
            tc.tile_pool(name="opool", bufs=1) as opool,
        ):
            A48 = const_pool.tile([128, N], bf16)
            B48 = const_pool.tile([128, J], bf16)
            QA5 = const_pool.tile([128, NT * 5], bf16)

            # Prewarm the ACT exp table set (~2.7us) during the DMA window.
            warm = const_pool.tile([1, 8], f32)
            kw = const_pool.tile([1, 8], bf16)
            nc.vector.memset(warm[:], -1.0)
            nc.scalar.activation(kw[:], warm[:], E)

            # Input DMAs, ordered so the first pair's dependencies land first.
            # All transfers are per-partition contiguous (no tiny-packet APs).
            nc.sync.dma_start(A48[:, 0:256], a_dram[:, 0:256])
            nc.sync.dma_start(B48[:], b_dram[:])
            nc.sync.dma_start(QA5[:], qa_dram[:])
            nc.sync.dma_start(A48[:, 256:2048], a_dram[:, 256:2048])
            nc.sync.dma_start(A48[:, 2048:N], a_dram[:, 2048:N])

            # col group g accumulates at partitions 32g..32g+4.  Nothing else
            # may write these banks: a start=True matmul clears has_written
            # for its partition rows across the whole bank (dropping real
            # partials), and a start=False one costs PSUM read-modify-write
            # activity that trips the PE power throttle.
            acc = apool.tile([101, J], f32)

            # arg tiles: a manually-managed ring of 3 persistent slots (2 PSUM
            # banks each).  Persistent tiles (not a cycling pool) make Tile
            # track dependencies at byte-range granularity: the spare cols
            # 864:1024 of each slot form a dependency-free scratch target for
            # HAM filler matmuls.  start=True is harmless here -- g is only
            # ever written with full-overwrite semantics.
            ring = [
                gring_pool.tile([128, 1024], f32, name=f"gslot{i}") for i in range(3)
            ]

            # HAM warmup: ~4us of back-to-back dummy matmuls during the DMA
            # window releases the PE clock throttle (1.2 -> 2.4 GHz) before
            # real work starts; the per-pair fillers below then keep it warm.
            dum = const_pool.tile([128, 256], bf16)
            nc.gpsimd.memset(dum[:], 0.5)
            for _ in range(12):
                nc.tensor.matmul(
                    ring[0][:, 864:1024], dum[:, 0:128], dum[:, 0:160],
                    start=True, stop=True, skip_group_check=True,
                )

            grp_last = {g: _grp_tiles(g)[-1] for g in range(NGRP)}
            kts: list = [None] * NT
            pending: list[int] = []  # tiles whose matmul2 is not yet emitted

            def emit_mm2(tiles):
                for j0, j1 in J_CHUNKS:
                    for t in tiles:
                        g = t % NGRP
                        k = kts[t]
                        rhs = k[:, j0:j1]
                        if k.tensor.dtype == mybir.dt.int16:
                            rhs = rhs.bitcast(bf16)
                        nc.tensor.matmul(
                            acc[32 * g : 32 * g + 5, j0:j1],
                            QA5[:, 5 * t : 5 * t + 5],
                            rhs,
                            start=(t == g),
                            stop=(t == grp_last[g]),
                            skip_group_check=True,
                            tile_position=(0, 32 * g),
                        )
                for t in tiles:
                    kts[t] = None

            for p in range(NT // 2):  # pair (2p, 2p+1): rows 0-63 / 64-127
                t0, t1 = 2 * p, 2 * p + 1
                g0 = ring[(2 * p) % 3]
                g1 = ring[(2 * p + 1) % 3]
                for j0, j1 in J_CHUNKS:
                    nc.tensor.matmul(
                        g0[:, j0:j1],
                        A48[0:64, t0 * 128 : (t0 + 1) * 128],
                        B48[0:64, j0:j1],
                        start=True,
                        stop=True,
                    )
                    nc.tensor.matmul(
                        g1[:, j0:j1],
                        A48[64:128, t1 * 128 : (t1 + 1) * 128],
                        B48[64:128, j0:j1],
                        start=True,
                        stop=True,
                    )
                # exp: ScalarE takes the even tile, VectorE the odd tile.
                # ScalarE is slightly faster per tile, so one pair mid-stream
                # goes all-ScalarE to balance total busy time (28/26 split).
                k0 = kpool.tile([128, J], bf16)
                nc.scalar.activation(k0[:], g0[:, 0:J], E)
                kts[t0] = k0
                if p == 13:
                    k1 = kpool.tile([128, J], bf16, name="k1a", tag="k1a", bufs=2)
                    nc.scalar.activation(k1[:], g1[:, 0:J], E)
                else:
                    k1 = kpool.tile([128, J], mybir.dt.int16, name="k1", tag="k1")
                    nc.vector.tensor_scalar(
                        k1[:], g1[:, 0:J], SCHRAUD_SCALE, SCHRAUD_BIAS, MU, AD
                    )
                kts[t1] = k1
                # HAM filler: the PE is only ~60% busy at the exp-paced
                # cadence, and the clock throttle re-engages unless the PE
                # stream is near gap-free.  Two throwaway matmuls per pair pad
                # the idle slivers; the ring-slot spare cols have no readers,
                # so these run whenever the PE would otherwise stall.
                for _ in range(2):
                    nc.tensor.matmul(
                        ring[(2 * p) % 3][:, 864:1024], dum[:, 0:128], dum[:, 0:160],
                        start=True, stop=True, skip_group_check=True,
                    )
                pending += [t0, t1]
                # Emit matmul2 for tiles at least 4 behind: their k tiles are
                # long since ready, so these never stall the PE queue (a
                # stalled matmul2 blocks the whole PE FIFO behind it, the PE
                # goes idle, and the clock throttle re-engages).
                if len(pending) >= 8:
                    emit_mm2(pending[:4])
                    pending = pending[4:]

            # Tail: pending == [48..53].  Col groups 2/3 finish with tiles
            # 50/51, so their copy-out and DMA overlap the final matmul2
            # batch for groups 0/1 (tiles 52/53) instead of serializing
            # after it.  Rows outside the four 5-row group blocks carry
            # garbage; the host fold only reads the group blocks.
            out_sb = opool.tile([101, J], f32)
            emit_mm2(pending[:4])
            pending = pending[4:]
            nc.scalar.copy(out_sb[64:101, :], acc[64:101, 0:J])
            nc.sync.dma_start(out_dram[64:101, :], out_sb[64:101, :])
            emit_mm2(pending)
            nc.scalar.copy(out_sb[0:37, :], acc[0:37, 0:J])
            nc.sync.dma_start(out_dram[0:37, :], out_sb[0:37, :])

    nc.compile()
    return nc


def _get_nc():
    nc = _BUILD_CACHE.get("v2")
    if nc is None:
        nc = _build_nc()
        _BUILD_CACHE["v2"] = nc
    return nc


def _split_bf16_3(a):
    import ml_dtypes

    bf = ml_dtypes.bfloat16
    a = np.asarray(a, dtype=np.float32)
    h = a.astype(bf)
    m = (a - h.astype(np.float32)).astype(bf)
    l = (a - h.astype(np.float32) - m.astype(np.float32)).astype(bf)
    return h, m, l


def _host_prep(q_in, image, v_alpha, v_beta):
    """Augmented feature matrices (fp32, O(N) work only)."""
    q_in = np.asarray(q_in, dtype=np.float32)
    image = np.asarray(image, dtype=np.float32)
    v_alpha = np.asarray(v_alpha, dtype=np.float32)
    v_beta = np.asarray(v_beta, dtype=np.float32)

    z = np.arange(D, dtype=np.float32)[:, None, None]
    y = np.arange(H, dtype=np.float32)[None, :, None]
    x = np.arange(W, dtype=np.float32)[None, None, :]
    shp = (D, H, W)
    zz = np.broadcast_to(v_alpha[0] * z / np.float32(SIGMA_ALPHA[0]), shp)
    xx = np.broadcast_to(v_alpha[1] * x / np.float32(SIGMA_ALPHA[1]), shp)
    yy = np.broadcast_to(v_alpha[2] * y / np.float32(SIGMA_ALPHA[2]), shp)
    xyz = np.stack([zz, yy, xx], axis=3)
    rgb = v_beta * np.transpose(image, (1, 2, 3, 0)) / np.float32(SIGMA_BETA)
    feats = np.concatenate([xyz, rgb], axis=3).reshape(-1, 6).astype(np.float32)

    # Center each feature dim: |f_i - f_j| is translation invariant, smaller
    # magnitudes mean less cancellation in the PE accumulation.
    feats = feats - (feats.min(axis=0) + feats.max(axis=0)) * np.float32(0.5)

    s = np.einsum("nf,nf->n", feats, feats).astype(np.float32)

    a_all = np.empty((F, N), dtype=np.float32)
    a_all[0:6] = feats.T
    a_all[6] = -0.5 * s
    a_all[7] = 1.0

    b_full = np.empty((F, N), dtype=np.float32)
    b_full[0:6] = feats.T
    b_full[6] = 1.0
    b_full[7] = -0.5 * s

    qa = np.empty((N, 3), dtype=np.float32)
    qa[:, 0] = q_in[0].reshape(-1)
    qa[:, 1] = q_in[1].reshape(-1)
    qa[:, 2] = 1.0
    return a_all, b_full, qa


def _in_maps(a_all, b_full, qa):
    """Per-core input dict: split/stacked bf16 layouts.

    a48: K=48 stacked splits [Ah;Ah;Ah;Am;Am;Al] in a 64-row strip; even
    tiles at partitions 0-63, odd tiles at 64-127 (row-paired matmul1).
    b48: [Bh;Bm;Bl;Bh;Bm;Bh] splits duplicated into both 64-row strips.
    qa5: [qh0,qh1,qm0,qm1,ones] per tile, on-chip layout [128, NT*5].
    """
    import ml_dtypes

    bf = ml_dtypes.bfloat16
    ah, am, al = _split_bf16_3(a_all)
    bh, bm, bl = _split_bf16_3(b_full)
    a48 = np.concatenate([ah, ah, ah, am, am, al], axis=0)  # [48, N]
    b48s = np.concatenate([bh, bm, bl, bh, bm, bh], axis=0)  # [48, N]

    a128 = np.zeros((128, N), dtype=bf)
    for t in range(NT):
        cs = slice(t * 128, (t + 1) * 128)
        a128[(t % 2) * 64 : (t % 2) * 64 + 48, cs] = a48[:, cs]

    qh = qa[:, 0:2].astype(bf)
    qm = (qa[:, 0:2] - qh.astype(np.float32)).astype(bf)
    qa5 = np.empty((N, 5), dtype=bf)
    qa5[:, 0:2] = qh
    qa5[:, 2:4] = qm
    qa5[:, 4] = np.float32(1.0)
    # on-chip layout: partition p, column 5t+c  <-  row t*128+p, col c
    qa5_chip = np.ascontiguousarray(
        qa5.reshape(NT, 128, 5).transpose(1, 0, 2).reshape(128, NT * 5)
    )

    maps = []
    for c in range(M_CORES):
        bslab = b48s[:, c * J : (c + 1) * J]
        b128 = np.zeros((128, J), dtype=bf)
        b128[0:48] = bslab
        b128[64:112] = bslab
        maps.append({"a48": a128, "b48": b128, "qa5": qa5_chip})
    return maps


def _fold_output(res):
    accs = []
    for c in range(M_CORES):
        o = res.results[c]["acc_out"]  # [101, J]; group g at rows 32g..32g+4
        accs.append(sum(o[32 * g : 32 * g + 5] for g in range(NGRP)))
    acc = np.concatenate(accs, axis=1)  # [5, N]
    filtered = acc[0:2] + acc[2:4]
    norm = acc[4]
    out = filtered / (norm[None, :] + EPS)
    return out.reshape(2, D, H, W).astype(np.float32)


def kernel(q_in, image, v_alpha, v_beta):
    a_all, b_full, qa = _host_prep(q_in, image, v_alpha, v_beta)
    nc = _get_nc()
    in_maps = _in_maps(a_all, b_full, qa)
    res = run_bass_kernel_spmd(nc, in_maps, core_ids=list(range(M_CORES)))
    return _fold_output(res)
